# revision 56
# baseline (speedup 1.0000x reference)
# Dense GAT layer (4 heads, dim 64) on Trainium2 via Bass/Tile.
#
# Math: h = x@W; e_ij = LeakyReLU(src_i + dst_j, 0.2); masked softmax over j
# with valid = adj & mask_i & mask_j; out = LN((alpha @ h) * mask_i).
#
# Key ideas:
#   * Host-side node compaction: rows/cols with mask==0 contribute nothing
#     (their output is just beta); gather the valid nodes on host, run the
#     kernel on the compacted [m, m] problem (padded to a multiple of 128),
#     scatter back.  Cuts n^2 elementwise work ~2.5x and all DMA traffic.
#   * exp(LeakyReLU(t)) = max(exp(t), exp(0.2 t)),  t = src_i + dst_j
#     exp(src_i + dst_j) = exp(src_i) * exp(dst_j)   (rank-1 separable)
#   * "e^T" orientation: j (softmax axis) on partitions, i on the free axis,
#     so alpha@h needs no transposes and rowsum is a matmul ones-column.
#   * Per-head route split: some heads compute exp(Prelu(t)) on the ACT
#     engine (replicated src row in PSUM + per-partition dst bias), others
#     use the separable max form on the DVE.  adj-mask multiply placement is
#     tunable between DVE and GPSIMD.
#   * rstd for LayerNorm = exp(-0.5*ln(var+eps)) so every ACT function used
#     (exp/parametric_relu/ln/copy) lives in ONE activation table set
#     (natural_log_exp_and_others) -> no ~2.7us table reloads.
#   * adjT/xT pre-transposed on host -> no on-device DMA transposes.
# Sharding: data-parallel, 2 graphs per core across 8 cores.

import numpy as np

H, D = 4, 64
HD = H * D
EPS = 1e-5
NCORES = 8

_PROG_CACHE = {}

# Tuning knobs (baked into the compiled program; cache key includes them).
CFG = dict(
    na=(5, 5, 0, 0),        # per head: number of j-chunks on the ACT route
    head_order=(0, 2, 1, 3),
    mask_gps=(),            # adj-mul on GPSIMD ((h,jc) pairs)
    osb_act=(0, 1),         # heads whose 1/rowsum scale runs on ACT
    rep_cast_act=True,      # arep/crep PSUM->f16 casts on ACT (else DVE)
    hones_act=True,         # hones copy on ACT (else DVE)
    b4op=True,              # B route: ts+ts+max instead of ts+stt
)


def _build_program(ng, NV, WI, KC, trivial_ln, cfg):
    import concourse.bacc as bacc
    import concourse.mybir as mybir
    import concourse.tile as tile
    from concourse.bass import ts

    f16 = mybir.dt.float16
    f32 = mybir.dt.float32
    AF = mybir.ActivationFunctionType
    OP = mybir.AluOpType

    n_v = NV * 128          # padded j extent (partition chunks)
    E = D + 1               # head block in hones (64 h cols + 1 ones col)
    in_dim = KC * 128
    # i-axis chunk list: [start, width] with a partial last chunk (WI <= n_v)
    ichunks = [(s, min(128, WI - s)) for s in range(0, WI, 128)]
    assert len(ichunks) == NV

    na = cfg["na"]
    head_order = cfg["head_order"]
    mask_gps = set(cfg["mask_gps"])
    osb_act = set(cfg["osb_act"])
    # matmul moving-operand pieces (<=512 psum f32 columns per instruction)
    pieces = [(s, min(512, n_v - s)) for s in range(0, n_v, 512)]

    nc = bacc.Bacc()

    # rep row blocks packed per cfg: srow for A-heads, arow+crow for B-heads
    rep_blocks = []
    for h_ in cfg["head_order"]:
        if cfg["na"][h_] > 0:
            rep_blocks.append(("s", h_))
        if cfg["na"][h_] < NV:
            rep_blocks.append(("a", h_))
            rep_blocks.append(("c", h_))
    R = len(rep_blocks)

    xT = nc.dram_tensor("xT", [ng, 128, KC * WI], f16, kind="ExternalInput")
    adjT = nc.dram_tensor("adjT", [ng, 128, NV * WI], f16,
                          kind="ExternalInput")
    wc = nc.dram_tensor("wc", [128, KC * HD], f16, kind="ExternalInput")
    rows_d = nc.dram_tensor("rows", [ng, 1, R * WI], f16,
                            kind="ExternalInput")
    small_d = nc.dram_tensor("small", [ng, 128, 3 * NV * H], f32,
                             kind="ExternalInput")
    if not trivial_ln:
        gam = nc.dram_tensor("gamma_rep", [128, HD], f32, kind="ExternalInput")
        bet = nc.dram_tensor("beta_rep", [128, HD], f32, kind="ExternalInput")
    out = nc.dram_tensor("out", [ng, 128, NV * HD], f16,
                         kind="ExternalOutput")

    from contextlib import ExitStack

    with tile.TileContext(nc) as tc, ExitStack() as ctx:
        def pool(**kw):
            return ctx.enter_context(tc.tile_pool(**kw))

        big = WI > 640          # scale pipeline depths down for large n
        consts = pool(name="consts", bufs=1)
        xt_pool = pool(name="xt", bufs=2)
        adjt_pool = pool(name="adjt", bufs=2)
        reps_pool = pool(name="reps", bufs=1 if big else 3)
        hones_pool = pool(name="hones", bufs=NV + 2)
        small_pool = pool(name="small", bufs=4)
        ew_pool = pool(name="ew", bufs=4 if big else 8)
        houter0 = NV + 3 <= 8
        u_pool = pool(
            name="u",
            bufs=(3 * NV + 2) if not big else
                 (H * NV + 2) if not houter0 else (NV + 2),
        )
        osb_pool = pool(name="osb", bufs=NV + 2)
        ln_pool = pool(name="ln", bufs=4)
        misc_pool = pool(name="misc", bufs=2 if big else 3)
        # PSUM (8 banks of 2KB): ph + pav.  For NV <= 5 keep one pav tile
        # per i-chunk alive across the head loop (PE overlaps elementwise);
        # for larger NV fall back to transient pav tiles with all heads' u
        # tiles held in SBUF.
        houter = NV + 3 <= 8
        ph_pool = pool(name="ph", bufs=3 if houter else 2, space="PSUM")
        pav_pool = pool(name="pav", bufs=NV if houter else 3, space="PSUM")

        # ---- pin the ACT spline table set that covers Exp/Prelu/Ln/Copy,
        # so the compiler never inserts mid-kernel ~1.3us table reloads ----
        from concourse.hw_specs import get_activation_tables

        tabs = list(get_activation_tables(nc.m.arch).items())
        set_id = next(
            i for i, (nm, fs) in enumerate(tabs)
            if {AF.Exp, AF.Prelu, AF.Ln, AF.Copy} <= fs
        )
        nc.scalar.add_instruction(
            mybir.InstLoadActFuncSet(
                name=nc.get_next_instruction_name(),
                act_func_set_id=set_id,
                ins=[],
                outs=[],
            )
        )

        # ---- constants ----
        wc_sb = consts.tile([128, KC * HD], f16, tag="wc")
        nc.sync.dma_start(wc_sb[:], wc[:])
        if not trivial_ln:
            gam_sb = consts.tile([128, HD], f32, tag="gam")
            nc.sync.dma_start(gam_sb[:], gam[:])
            bet_sb = consts.tile([128, HD], f32, tag="bet")
            nc.sync.dma_start(bet_sb[:], bet[:])
        eps_sb = consts.tile([128, 1], f32, tag="eps")
        nc.vector.memset(eps_sb[:], EPS)

        need_a = any(na[h] > 0 for h in range(H))
        need_b = any(na[h] < NV for h in range(H))

        for g in range(ng):
            # ---- input DMAs (host pre-transposed; plain loads) ----
            # ---- batched input DMAs: one transfer each for the small
            # tables, x, the broadcast row-replicas, and the adjacency ----
            smalls = small_pool.tile([128, 3 * NV * H], f32, tag="smalls")
            nc.sync.dma_start(smalls[:], small_d[g])
            bmall = smalls[:, 0 : NV * H]
            dmall = smalls[:, NV * H : 2 * NV * H]
            dcall = smalls[:, 2 * NV * H : 3 * NV * H]
            xt_all = xt_pool.tile([128, KC * WI], f16, tag="xt")
            for kc in range(KC):
                nc.sync.dma_start(
                    xt_all[:, kc * WI : (kc + 1) * WI],
                    xT[g, :, kc * WI : (kc + 1) * WI],
                )
            xt = [xt_all[:, kc * WI : (kc + 1) * WI] for kc in range(KC)]
            # all row replicas in one partition-broadcast DMA read from HBM
            reps_all = reps_pool.tile([128, R * WI], f16, tag="reps")
            nc.sync.dma_start(
                reps_all[:], rows_d[g].to_broadcast([128, R * WI])
            )
            sreps = {}
            areps = {}
            creps = {}
            for bi, (kind, h_) in enumerate(rep_blocks):
                v = reps_all[:, bi * WI : (bi + 1) * WI]
                (sreps if kind == "s" else areps if kind == "a" else creps)[
                    h_
                ] = v
            adjt_all = adjt_pool.tile([128, NV * WI], f16, tag="adjt")
            nc.sync.dma_start(adjt_all[:], adjT[g])
            adjt = [
                adjt_all[:, jc * WI : (jc + 1) * WI] for jc in range(NV)
            ]


            # ---- h_ext per chunk: hones (fp16 h + ones col), dst scalars ----
            # (after the rep matmuls so the serialized ph ring does not
            # head-of-line-block the PE queue for the replicates)
            hones = []
            for ic, (istart, icw) in enumerate(ichunks):
                ph = ph_pool.tile([128, HD], f32, tag="ph")
                for kc in range(KC):
                    nc.tensor.matmul(
                        ph[0:icw],
                        xt[kc][:, istart : istart + icw],
                        wc_sb[:, ts(kc, HD)],
                        start=(kc == 0),
                        stop=(kc == KC - 1),
                    )
                ho = hones_pool.tile([128, H * E], f16, tag="hones")
                ho3 = ho[:].rearrange("p (h e) -> p h e", h=H)
                if icw < 128:
                    # zero the padded j rows so pav contractions stay finite
                    nc.vector.memset(ho[icw:128, :], 0.0)
                if cfg["hones_act"]:
                    nc.scalar.copy(
                        ho3[0:icw, :, 0:D],
                        ph[0:icw].rearrange("p (h d) -> p h d", h=H),
                    )
                else:
                    nc.vector.tensor_copy(
                        ho3[0:icw, :, 0:D],
                        ph[0:icw].rearrange("p (h d) -> p h d", h=H),
                    )
                nc.vector.memset(ho3[0:icw, :, D : D + 1], 1.0)
                hones.append(ho)

            # ---- elementwise u tiles + per-head alpha@h accumulation ----
            o_sb = [
                osb_pool.tile([128, HD], f32, tag="osb", name=f"osb_{g}_{i}")
                for i in range(NV)
            ]
            o2all = misc_pool.tile(
                [128, NV * HD], f16, tag="o2all", name=f"o2all_{g}"
            )
            pav = None
            if houter:
                pav = [
                    pav_pool.tile(
                        [128, H * E], f32, tag="pav", name=f"pav_{g}_{i}"
                    )
                    for i in range(NV)
                ]
            u_save = {}
            for h in head_order:
                nah = na[h]
                a_jcs = list(range(nah))
                b_jcs = list(range(nah, NV))

                srep = sreps.get(h)
                arep = areps.get(h)
                crep = creps.get(h)

                u_tiles = [None] * NV
                for jc in a_jcs:
                    lrt = ew_pool.tile([128, WI], f16, tag="lrt")
                    nc.scalar.activation(
                        lrt[:], srep, AF.Prelu,
                        bias=dcall[:, jc * H + h : jc * H + h + 1],
                        alpha=0.2,
                    )
                    up = ew_pool.tile([128, WI], f16, tag="up")
                    nc.scalar.activation(up[:], lrt[:], AF.Exp)
                    u = u_pool.tile([128, WI], f16, tag="u")
                    meng = nc.gpsimd if (h, jc) in mask_gps else nc.vector
                    meng.tensor_mul(u[:], up[:], adjt[jc])
                    u_tiles[jc] = u[:]
                for jc in b_jcs:
                    t2 = ew_pool.tile([128, WI], f16, tag="t2")
                    nc.vector.tensor_scalar(
                        t2[:], crep,
                        dmall[:, jc * H + h : jc * H + h + 1], None,
                        op0=OP.mult,
                    )
                    w = ew_pool.tile([128, WI], f16, tag="w")
                    if cfg["b4op"]:
                        t1 = ew_pool.tile([128, WI], f16, tag="t1")
                        nc.vector.tensor_scalar(
                            t1[:], arep,
                            bmall[:, jc * H + h : jc * H + h + 1], None,
                            op0=OP.mult,
                        )
                        nc.vector.tensor_max(w[:], t1[:], t2[:])
                    else:
                        nc.vector.scalar_tensor_tensor(
                            w[:], arep,
                            bmall[:, jc * H + h : jc * H + h + 1], t2[:],
                            op0=OP.mult, op1=OP.max,
                        )
                    u = u_pool.tile([128, WI], f16, tag="u")
                    meng = nc.gpsimd if (h, jc) in mask_gps else nc.vector
                    meng.tensor_mul(u[:], w[:], adjt[jc])
                    u_tiles[jc] = u[:]

                if houter:
                    # alpha@h: this head's block of every chunk's PSUM tile,
                    # so the PE works during the elementwise phase
                    for ic, (istart, icw) in enumerate(ichunks):
                        for jc in range(NV):
                            nc.tensor.matmul(
                                pav[ic][0:icw, ts(h, E)],
                                u_tiles[jc][:, istart : istart + icw],
                                hones[jc][:, ts(h, E)],
                                start=(jc == 0),
                                stop=(jc == NV - 1),
                            )
                else:
                    for jc in range(NV):
                        u_save[(h, jc)] = u_tiles[jc]

            # ---- per-chunk softmax-normalize, LN stats, LN apply, store.
            # rstd = exp(-0.5 * ln(var + eps)); ln/exp share the table set
            # with Prelu/Exp above, so no ACT table reloads.
            for ic, (istart, icw) in enumerate(ichunks):
                if houter:
                    pav_t = pav[ic]
                else:
                    pav_t = pav_pool.tile([128, H * E], f32, tag="pav")
                    for h in range(H):
                        for jc in range(NV):
                            nc.tensor.matmul(
                                pav_t[0:icw, ts(h, E)],
                                u_save[(h, jc)][:, istart : istart + icw],
                                hones[jc][:, ts(h, E)],
                                start=(jc == 0),
                                stop=(jc == NV - 1),
                            )
                pav3 = pav_t[0:icw].rearrange("p (h e) -> p h e", h=H)
                rs4 = ln_pool.tile([128, H], f32, tag="rs4")
                nc.vector.reciprocal(rs4[0:icw], pav3[:, :, D])
                for hh in range(H):
                    if hh in osb_act:
                        nc.scalar.mul(
                            o_sb[ic][0:icw, ts(hh, D)],
                            pav3[:, hh, 0:D],
                            rs4[0:icw, hh : hh + 1],
                        )
                    else:
                        nc.vector.tensor_scalar(
                            o_sb[ic][0:icw, ts(hh, D)],
                            pav3[:, hh, 0:D],
                            rs4[0:icw, hh : hh + 1],
                            None,
                            op0=OP.mult,
                        )
                st6 = ln_pool.tile([128, 6], f32, tag="st6")
                nc.vector.bn_stats(st6[0:icw], o_sb[ic][0:icw])
                mv = ln_pool.tile([128, 2], f32, tag="mv")
                nc.vector.bn_aggr(mv[0:icw], st6[0:icw])
                rstd = ln_pool.tile([128, 2], f32, tag="rstd")
                nc.scalar.activation(
                    rstd[0:icw, 0:1], mv[0:icw, 1:2], AF.Ln,
                    bias=eps_sb[0:icw],
                )
                nc.scalar.activation(
                    rstd[0:icw, 1:2], rstd[0:icw, 0:1], AF.Exp, scale=-0.5
                )
                nmr = ln_pool.tile([128, 1], f32, tag="nmr")
                nc.vector.scalar_tensor_tensor(
                    nmr[0:icw], mv[0:icw, 0:1], -1.0, rstd[0:icw, 1:2],
                    op0=OP.mult, op1=OP.mult,
                )
                if trivial_ln:
                    nc.scalar.activation(
                        o2all[0:icw, ts(ic, HD)],
                        o_sb[ic][0:icw],
                        AF.Identity,
                        bias=nmr[0:icw],
                        scale=rstd[0:icw, 1:2],
                    )
                else:
                    o3 = misc_pool.tile([128, HD], f32, tag="o3")
                    nc.scalar.activation(
                        o3[0:icw],
                        o_sb[ic][0:icw],
                        AF.Identity,
                        bias=nmr[0:icw],
                        scale=rstd[0:icw, 1:2],
                    )
                    nc.vector.tensor_mul(o3[0:icw], o3[0:icw], gam_sb[0:icw])
                    nc.vector.tensor_add(
                        o2all[0:icw, ts(ic, HD)], o3[0:icw], bet_sb[0:icw]
                    )
                nc.gpsimd.dma_start(
                    out[g, 0:icw, ic * HD : (ic + 1) * HD],
                    o2all[0:icw, ts(ic, HD)],
                )


    nc.compile()
    return nc


def _host_prep(x, adj, mask, W, a_src, a_dst, gamma, beta, ng, NV, WI, idxs):
    """Per-core input maps: compaction + dtype packing + weight folding."""
    b, n, in_dim = x.shape
    KC = in_dim // 128
    n_v = NV * 128

    # Fold attention vectors into W:  Wa[c, h] = sum_d W[c, h*D+d] * a[h, d]
    Wr = W.astype(np.float64).reshape(in_dim, H, D)
    wa_src = np.einsum("chd,hd->ch", Wr, a_src.astype(np.float64))
    wa_dst = np.einsum("chd,hd->ch", Wr, a_dst.astype(np.float64))

    wc_full = np.ascontiguousarray(
        W.astype(np.float16).reshape(KC, 128, HD).transpose(1, 0, 2)
    ).reshape(128, KC * HD)
    x16 = x.astype(np.float16)
    adj01 = adj != 0
    NV = n_v // 128

    # rep row packing must mirror _build_program's rep_blocks
    rep_blocks = []
    for h_ in CFG["head_order"]:
        if CFG["na"][h_] > 0:
            rep_blocks.append(("s", h_))
        if CFG["na"][h_] < NV:
            rep_blocks.append(("a", h_))
            rep_blocks.append(("c", h_))
    R = len(rep_blocks)

    in_maps = []
    for c in range(NCORES):
        xT = np.zeros((ng, 128, KC * WI), np.float16)
        adjTc = np.zeros((ng, 128, NV * WI), np.float16)
        rows = np.zeros((ng, 1, R * WI), np.float16)
        small = np.zeros((ng, 128, 3 * NV * H), np.float32)
        for gl in range(ng):
            g = c * ng + gl
            idx = idxs[g]
            m = len(idx)
            xc = x[g][idx].astype(np.float64)          # [m, in_dim]
            # xT[p, kc*WI + i] = x[idx[i], kc*128 + p]
            xt_f = np.zeros((WI, in_dim), np.float16)
            xt_f[:m] = x16[g][idx]
            xT[gl] = (
                xt_f.reshape(WI, KC, 128).transpose(2, 1, 0).reshape(
                    128, KC * WI
                )
            )
            # adjT[p, jc*WI + i] = adj[idx[i], idx[jc*128+p]]
            at = np.zeros((n_v, WI), np.float16)
            at[:m, :m] = adj01[g][np.ix_(idx, idx)].T
            adjTc[gl] = at.reshape(NV, 128, WI).transpose(1, 0, 2).reshape(
                128, NV * WI
            )
            srcv = np.zeros((WI, H))
            srcv[:m] = xc @ wa_src                     # [m, H]
            dstv = np.zeros((n_v, H))
            dstv[:m] = xc @ wa_dst
            for bi, (kind, h_) in enumerate(rep_blocks):
                if kind == "s":
                    v = srcv[:, h_]
                elif kind == "a":
                    v = np.exp(srcv[:, h_])
                else:
                    v = np.exp(0.2 * srcv[:, h_])
                rows[gl, 0, bi * WI : (bi + 1) * WI] = v.astype(np.float16)
            # dcall[p, c*H + h] = dst[c*128 + p, h]
            dcall = dstv.reshape(NV, 128, H).transpose(1, 0, 2).reshape(
                128, NV * H
            )
            small[gl, :, 0 : NV * H] = np.exp(dcall)
            small[gl, :, NV * H : 2 * NV * H] = np.exp(0.2 * dcall)
            small[gl, :, 2 * NV * H :] = dcall
        m_map = {
            "xT": xT,
            "adjT": adjTc,
            "wc": wc_full,
            "rows": rows,
            "small": small,
        }
        if not (np.all(gamma == 1.0) and np.all(beta == 0.0)):
            m_map["gamma_rep"] = np.ascontiguousarray(
                np.broadcast_to(gamma.astype(np.float32), (128, HD))
            )
            m_map["beta_rep"] = np.ascontiguousarray(
                np.broadcast_to(beta.astype(np.float32), (128, HD))
            )
        in_maps.append(m_map)
    return in_maps


def kernel(x, adj, mask, W, a_src, a_dst, gamma, beta, _trace=False):
    from concourse.bass_utils import run_bass_kernel_spmd

    b, n, in_dim = x.shape
    ng = b // NCORES
    trivial_ln = bool(np.all(gamma == 1.0) and np.all(beta == 0.0))

    idxs = [np.nonzero(mask[g] > 0)[0] for g in range(b)]
    max_m = max((len(i) for i in idxs), default=1)
    NV = max(1, -(-max_m // 128))
    WI = max(128, -(-max_m // 64) * 64)
    KC = in_dim // 128

    key = (ng, NV, WI, KC, trivial_ln, repr(sorted(CFG.items())))
    if key not in _PROG_CACHE:
        _PROG_CACHE[key] = _build_program(ng, NV, WI, KC, trivial_ln, CFG)
    nc = _PROG_CACHE[key]

    in_maps = _host_prep(
        x, adj, mask, W, a_src, a_dst, gamma, beta, ng, NV, WI, idxs
    )
    res = run_bass_kernel_spmd(
        nc, in_maps, core_ids=list(range(NCORES)), trace=_trace
    )
    full = np.zeros((b, n, HD), np.float32)
    if not trivial_ln:
        full[:] = beta.astype(np.float32)[None, None, :]
    for c in range(NCORES):
        o = res.results[c]["out"].reshape(ng, 128, NV, HD)
        o = o.transpose(0, 2, 1, 3).reshape(ng, NV * 128, HD)[:, :WI]
        for gl in range(ng):
            g = c * ng + gl
            idx = idxs[g]
            full[g, idx] = o[gl, : len(idx)].astype(np.float32)
    if _trace:
        return full, res
    return full


# revision 57
# speedup vs baseline: 1.0062x; 1.0062x over previous
# Dense GAT layer (4 heads, dim 64) on Trainium2 via Bass/Tile.
#
# Math: h = x@W; e_ij = LeakyReLU(src_i + dst_j, 0.2); masked softmax over j
# with valid = adj & mask_i & mask_j; out = LN((alpha @ h) * mask_i).
#
# Key ideas:
#   * Host-side node compaction: rows/cols with mask==0 contribute nothing
#     (their output is just beta); gather the valid nodes on host, run the
#     kernel on the compacted [m, m] problem (padded to a multiple of 128),
#     scatter back.  Cuts n^2 elementwise work ~2.5x and all DMA traffic.
#   * exp(LeakyReLU(t)) = max(exp(t), exp(0.2 t)),  t = src_i + dst_j
#     exp(src_i + dst_j) = exp(src_i) * exp(dst_j)   (rank-1 separable)
#   * "e^T" orientation: j (softmax axis) on partitions, i on the free axis,
#     so alpha@h needs no transposes and rowsum is a matmul ones-column.
#   * Per-head route split: some heads compute exp(Prelu(t)) on the ACT
#     engine (replicated src row in PSUM + per-partition dst bias), others
#     use the separable max form on the DVE.  adj-mask multiply placement is
#     tunable between DVE and GPSIMD.
#   * rstd for LayerNorm = exp(-0.5*ln(var+eps)) so every ACT function used
#     (exp/parametric_relu/ln/copy) lives in ONE activation table set
#     (natural_log_exp_and_others) -> no ~2.7us table reloads.
#   * adjT/xT pre-transposed on host -> no on-device DMA transposes.
# Sharding: data-parallel, 2 graphs per core across 8 cores.

import numpy as np

H, D = 4, 64
HD = H * D
EPS = 1e-5
NCORES = 8

_PROG_CACHE = {}

# Tuning knobs (baked into the compiled program; cache key includes them).
CFG = dict(
    na=(5, 5, 0, 0),        # per head: number of j-chunks on the ACT route
    head_order=(2, 0, 3, 1),
    mask_gps=(),            # adj-mul on GPSIMD ((h,jc) pairs)
    osb_act=(0, 1),         # heads whose 1/rowsum scale runs on ACT
    rep_cast_act=True,      # arep/crep PSUM->f16 casts on ACT (else DVE)
    hones_act=True,         # hones copy on ACT (else DVE)
    b4op=True,              # B route: ts+ts+max instead of ts+stt
)


def _build_program(ng, NV, WI, KC, trivial_ln, cfg):
    import concourse.bacc as bacc
    import concourse.mybir as mybir
    import concourse.tile as tile
    from concourse.bass import ts

    f16 = mybir.dt.float16
    f32 = mybir.dt.float32
    AF = mybir.ActivationFunctionType
    OP = mybir.AluOpType

    n_v = NV * 128          # padded j extent (partition chunks)
    E = D + 1               # head block in hones (64 h cols + 1 ones col)
    in_dim = KC * 128
    # i-axis chunk list: [start, width] with a partial last chunk (WI <= n_v)
    ichunks = [(s, min(128, WI - s)) for s in range(0, WI, 128)]
    assert len(ichunks) == NV

    na = cfg["na"]
    head_order = cfg["head_order"]
    mask_gps = set(cfg["mask_gps"])
    osb_act = set(cfg["osb_act"])
    # matmul moving-operand pieces (<=512 psum f32 columns per instruction)
    pieces = [(s, min(512, n_v - s)) for s in range(0, n_v, 512)]

    nc = bacc.Bacc()

    # rep row blocks packed per cfg: srow for A-heads, arow+crow for B-heads
    rep_blocks = []
    for h_ in cfg["head_order"]:
        if cfg["na"][h_] > 0:
            rep_blocks.append(("s", h_))
        if cfg["na"][h_] < NV:
            rep_blocks.append(("a", h_))
            rep_blocks.append(("c", h_))
    R = len(rep_blocks)

    xT = nc.dram_tensor("xT", [ng, 128, KC * WI], f16, kind="ExternalInput")
    adjT = nc.dram_tensor("adjT", [ng, 128, NV * WI], f16,
                          kind="ExternalInput")
    wc = nc.dram_tensor("wc", [128, KC * HD], f16, kind="ExternalInput")
    rows_d = nc.dram_tensor("rows", [ng, 1, R * WI], f16,
                            kind="ExternalInput")
    small_d = nc.dram_tensor("small", [ng, 128, 3 * NV * H], f32,
                             kind="ExternalInput")
    if not trivial_ln:
        gam = nc.dram_tensor("gamma_rep", [128, HD], f32, kind="ExternalInput")
        bet = nc.dram_tensor("beta_rep", [128, HD], f32, kind="ExternalInput")
    out = nc.dram_tensor("out", [ng, 128, NV * HD], f16,
                         kind="ExternalOutput")

    from contextlib import ExitStack

    with tile.TileContext(nc) as tc, ExitStack() as ctx:
        def pool(**kw):
            return ctx.enter_context(tc.tile_pool(**kw))

        big = WI > 640          # scale pipeline depths down for large n
        consts = pool(name="consts", bufs=1)
        xt_pool = pool(name="xt", bufs=2)
        adjt_pool = pool(name="adjt", bufs=2)
        reps_pool = pool(name="reps", bufs=1 if big else 3)
        hones_pool = pool(name="hones", bufs=NV + 2)
        small_pool = pool(name="small", bufs=4)
        ew_pool = pool(name="ew", bufs=4 if big else 8)
        houter0 = NV + 3 <= 8
        u_pool = pool(
            name="u",
            bufs=(3 * NV + 2) if not big else
                 (H * NV + 2) if not houter0 else (NV + 2),
        )
        osb_pool = pool(name="osb", bufs=NV + 2)
        ln_pool = pool(name="ln", bufs=4)
        misc_pool = pool(name="misc", bufs=2 if big else 3)
        # PSUM (8 banks of 2KB): ph + pav.  For NV <= 5 keep one pav tile
        # per i-chunk alive across the head loop (PE overlaps elementwise);
        # for larger NV fall back to transient pav tiles with all heads' u
        # tiles held in SBUF.
        houter = NV + 3 <= 8
        ph_pool = pool(name="ph", bufs=3 if houter else 2, space="PSUM")
        pav_pool = pool(name="pav", bufs=NV if houter else 3, space="PSUM")

        # ---- pin the ACT spline table set that covers Exp/Prelu/Ln/Copy,
        # so the compiler never inserts mid-kernel ~1.3us table reloads ----
        from concourse.hw_specs import get_activation_tables

        tabs = list(get_activation_tables(nc.m.arch).items())
        set_id = next(
            i for i, (nm, fs) in enumerate(tabs)
            if {AF.Exp, AF.Prelu, AF.Ln, AF.Copy} <= fs
        )
        nc.scalar.add_instruction(
            mybir.InstLoadActFuncSet(
                name=nc.get_next_instruction_name(),
                act_func_set_id=set_id,
                ins=[],
                outs=[],
            )
        )

        # ---- constants ----
        wc_sb = consts.tile([128, KC * HD], f16, tag="wc")
        nc.sync.dma_start(wc_sb[:], wc[:])
        if not trivial_ln:
            gam_sb = consts.tile([128, HD], f32, tag="gam")
            nc.sync.dma_start(gam_sb[:], gam[:])
            bet_sb = consts.tile([128, HD], f32, tag="bet")
            nc.sync.dma_start(bet_sb[:], bet[:])
        eps_sb = consts.tile([128, 1], f32, tag="eps")
        nc.vector.memset(eps_sb[:], EPS)

        need_a = any(na[h] > 0 for h in range(H))
        need_b = any(na[h] < NV for h in range(H))

        for g in range(ng):
            # ---- input DMAs (host pre-transposed; plain loads) ----
            # ---- batched input DMAs: one transfer each for the small
            # tables, x, the broadcast row-replicas, and the adjacency ----
            smalls = small_pool.tile([128, 3 * NV * H], f32, tag="smalls")
            nc.sync.dma_start(smalls[:], small_d[g])
            bmall = smalls[:, 0 : NV * H]
            dmall = smalls[:, NV * H : 2 * NV * H]
            dcall = smalls[:, 2 * NV * H : 3 * NV * H]
            xt_all = xt_pool.tile([128, KC * WI], f16, tag="xt")
            for kc in range(KC):
                nc.sync.dma_start(
                    xt_all[:, kc * WI : (kc + 1) * WI],
                    xT[g, :, kc * WI : (kc + 1) * WI],
                )
            xt = [xt_all[:, kc * WI : (kc + 1) * WI] for kc in range(KC)]
            # all row replicas in one partition-broadcast DMA read from HBM
            reps_all = reps_pool.tile([128, R * WI], f16, tag="reps")
            nc.sync.dma_start(
                reps_all[:], rows_d[g].to_broadcast([128, R * WI])
            )
            sreps = {}
            areps = {}
            creps = {}
            for bi, (kind, h_) in enumerate(rep_blocks):
                v = reps_all[:, bi * WI : (bi + 1) * WI]
                (sreps if kind == "s" else areps if kind == "a" else creps)[
                    h_
                ] = v
            adjt_all = adjt_pool.tile([128, NV * WI], f16, tag="adjt")
            nc.sync.dma_start(adjt_all[:], adjT[g])
            adjt = [
                adjt_all[:, jc * WI : (jc + 1) * WI] for jc in range(NV)
            ]


            # ---- h_ext per chunk: hones (fp16 h + ones col), dst scalars ----
            # (after the rep matmuls so the serialized ph ring does not
            # head-of-line-block the PE queue for the replicates)
            hones = []
            for ic, (istart, icw) in enumerate(ichunks):
                ph = ph_pool.tile([128, HD], f32, tag="ph")
                for kc in range(KC):
                    nc.tensor.matmul(
                        ph[0:icw],
                        xt[kc][:, istart : istart + icw],
                        wc_sb[:, ts(kc, HD)],
                        start=(kc == 0),
                        stop=(kc == KC - 1),
                    )
                ho = hones_pool.tile([128, H * E], f16, tag="hones")
                ho3 = ho[:].rearrange("p (h e) -> p h e", h=H)
                if icw < 128:
                    # zero the padded j rows so pav contractions stay finite
                    nc.vector.memset(ho[icw:128, :], 0.0)
                if cfg["hones_act"]:
                    nc.scalar.copy(
                        ho3[0:icw, :, 0:D],
                        ph[0:icw].rearrange("p (h d) -> p h d", h=H),
                    )
                else:
                    nc.vector.tensor_copy(
                        ho3[0:icw, :, 0:D],
                        ph[0:icw].rearrange("p (h d) -> p h d", h=H),
                    )
                nc.vector.memset(ho3[0:icw, :, D : D + 1], 1.0)
                hones.append(ho)

            # ---- elementwise u tiles + per-head alpha@h accumulation ----
            o_sb = [
                osb_pool.tile([128, HD], f32, tag="osb", name=f"osb_{g}_{i}")
                for i in range(NV)
            ]
            o2all = misc_pool.tile(
                [128, NV * HD], f16, tag="o2all", name=f"o2all_{g}"
            )
            pav = None
            if houter:
                pav = [
                    pav_pool.tile(
                        [128, H * E], f32, tag="pav", name=f"pav_{g}_{i}"
                    )
                    for i in range(NV)
                ]
            u_save = {}
            for h in head_order:
                nah = na[h]
                a_jcs = list(range(nah))
                b_jcs = list(range(nah, NV))

                srep = sreps.get(h)
                arep = areps.get(h)
                crep = creps.get(h)

                u_tiles = [None] * NV
                for jc in a_jcs:
                    lrt = ew_pool.tile([128, WI], f16, tag="lrt")
                    nc.scalar.activation(
                        lrt[:], srep, AF.Prelu,
                        bias=dcall[:, jc * H + h : jc * H + h + 1],
                        alpha=0.2,
                    )
                    up = ew_pool.tile([128, WI], f16, tag="up")
                    nc.scalar.activation(up[:], lrt[:], AF.Exp)
                    u = u_pool.tile([128, WI], f16, tag="u")
                    meng = nc.gpsimd if (h, jc) in mask_gps else nc.vector
                    meng.tensor_mul(u[:], up[:], adjt[jc])
                    u_tiles[jc] = u[:]
                for jc in b_jcs:
                    t2 = ew_pool.tile([128, WI], f16, tag="t2")
                    nc.vector.tensor_scalar(
                        t2[:], crep,
                        dmall[:, jc * H + h : jc * H + h + 1], None,
                        op0=OP.mult,
                    )
                    w = ew_pool.tile([128, WI], f16, tag="w")
                    if cfg["b4op"]:
                        t1 = ew_pool.tile([128, WI], f16, tag="t1")
                        nc.vector.tensor_scalar(
                            t1[:], arep,
                            bmall[:, jc * H + h : jc * H + h + 1], None,
                            op0=OP.mult,
                        )
                        nc.vector.tensor_max(w[:], t1[:], t2[:])
                    else:
                        nc.vector.scalar_tensor_tensor(
                            w[:], arep,
                            bmall[:, jc * H + h : jc * H + h + 1], t2[:],
                            op0=OP.mult, op1=OP.max,
                        )
                    u = u_pool.tile([128, WI], f16, tag="u")
                    meng = nc.gpsimd if (h, jc) in mask_gps else nc.vector
                    meng.tensor_mul(u[:], w[:], adjt[jc])
                    u_tiles[jc] = u[:]

                if houter:
                    # alpha@h: this head's block of every chunk's PSUM tile,
                    # so the PE works during the elementwise phase
                    for ic, (istart, icw) in enumerate(ichunks):
                        for jc in range(NV):
                            nc.tensor.matmul(
                                pav[ic][0:icw, ts(h, E)],
                                u_tiles[jc][:, istart : istart + icw],
                                hones[jc][:, ts(h, E)],
                                start=(jc == 0),
                                stop=(jc == NV - 1),
                            )
                else:
                    for jc in range(NV):
                        u_save[(h, jc)] = u_tiles[jc]

            # ---- per-chunk softmax-normalize, LN stats, LN apply, store.
            # rstd = exp(-0.5 * ln(var + eps)); ln/exp share the table set
            # with Prelu/Exp above, so no ACT table reloads.
            for ic, (istart, icw) in enumerate(ichunks):
                if houter:
                    pav_t = pav[ic]
                else:
                    pav_t = pav_pool.tile([128, H * E], f32, tag="pav")
                    for h in range(H):
                        for jc in range(NV):
                            nc.tensor.matmul(
                                pav_t[0:icw, ts(h, E)],
                                u_save[(h, jc)][:, istart : istart + icw],
                                hones[jc][:, ts(h, E)],
                                start=(jc == 0),
                                stop=(jc == NV - 1),
                            )
                pav3 = pav_t[0:icw].rearrange("p (h e) -> p h e", h=H)
                rs4 = ln_pool.tile([128, H], f32, tag="rs4")
                nc.vector.reciprocal(rs4[0:icw], pav3[:, :, D])
                for hh in range(H):
                    if hh in osb_act:
                        nc.scalar.mul(
                            o_sb[ic][0:icw, ts(hh, D)],
                            pav3[:, hh, 0:D],
                            rs4[0:icw, hh : hh + 1],
                        )
                    else:
                        nc.vector.tensor_scalar(
                            o_sb[ic][0:icw, ts(hh, D)],
                            pav3[:, hh, 0:D],
                            rs4[0:icw, hh : hh + 1],
                            None,
                            op0=OP.mult,
                        )
                st6 = ln_pool.tile([128, 6], f32, tag="st6")
                nc.vector.bn_stats(st6[0:icw], o_sb[ic][0:icw])
                mv = ln_pool.tile([128, 2], f32, tag="mv")
                nc.vector.bn_aggr(mv[0:icw], st6[0:icw])
                rstd = ln_pool.tile([128, 2], f32, tag="rstd")
                nc.scalar.activation(
                    rstd[0:icw, 0:1], mv[0:icw, 1:2], AF.Ln,
                    bias=eps_sb[0:icw],
                )
                nc.scalar.activation(
                    rstd[0:icw, 1:2], rstd[0:icw, 0:1], AF.Exp, scale=-0.5
                )
                nmr = ln_pool.tile([128, 1], f32, tag="nmr")
                nc.vector.scalar_tensor_tensor(
                    nmr[0:icw], mv[0:icw, 0:1], -1.0, rstd[0:icw, 1:2],
                    op0=OP.mult, op1=OP.mult,
                )
                if trivial_ln:
                    nc.scalar.activation(
                        o2all[0:icw, ts(ic, HD)],
                        o_sb[ic][0:icw],
                        AF.Identity,
                        bias=nmr[0:icw],
                        scale=rstd[0:icw, 1:2],
                    )
                else:
                    o3 = misc_pool.tile([128, HD], f32, tag="o3")
                    nc.scalar.activation(
                        o3[0:icw],
                        o_sb[ic][0:icw],
                        AF.Identity,
                        bias=nmr[0:icw],
                        scale=rstd[0:icw, 1:2],
                    )
                    nc.vector.tensor_mul(o3[0:icw], o3[0:icw], gam_sb[0:icw])
                    nc.vector.tensor_add(
                        o2all[0:icw, ts(ic, HD)], o3[0:icw], bet_sb[0:icw]
                    )
                nc.gpsimd.dma_start(
                    out[g, 0:icw, ic * HD : (ic + 1) * HD],
                    o2all[0:icw, ts(ic, HD)],
                )


    nc.compile()
    return nc


def _host_prep(x, adj, mask, W, a_src, a_dst, gamma, beta, ng, NV, WI, idxs):
    """Per-core input maps: compaction + dtype packing + weight folding."""
    b, n, in_dim = x.shape
    KC = in_dim // 128
    n_v = NV * 128

    # Fold attention vectors into W:  Wa[c, h] = sum_d W[c, h*D+d] * a[h, d]
    Wr = W.astype(np.float64).reshape(in_dim, H, D)
    wa_src = np.einsum("chd,hd->ch", Wr, a_src.astype(np.float64))
    wa_dst = np.einsum("chd,hd->ch", Wr, a_dst.astype(np.float64))

    wc_full = np.ascontiguousarray(
        W.astype(np.float16).reshape(KC, 128, HD).transpose(1, 0, 2)
    ).reshape(128, KC * HD)
    x16 = x.astype(np.float16)
    adj01 = adj != 0
    NV = n_v // 128

    # rep row packing must mirror _build_program's rep_blocks
    rep_blocks = []
    for h_ in CFG["head_order"]:
        if CFG["na"][h_] > 0:
            rep_blocks.append(("s", h_))
        if CFG["na"][h_] < NV:
            rep_blocks.append(("a", h_))
            rep_blocks.append(("c", h_))
    R = len(rep_blocks)

    in_maps = []
    for c in range(NCORES):
        xT = np.zeros((ng, 128, KC * WI), np.float16)
        adjTc = np.zeros((ng, 128, NV * WI), np.float16)
        rows = np.zeros((ng, 1, R * WI), np.float16)
        small = np.zeros((ng, 128, 3 * NV * H), np.float32)
        for gl in range(ng):
            g = c * ng + gl
            idx = idxs[g]
            m = len(idx)
            xc = x[g][idx].astype(np.float64)          # [m, in_dim]
            # xT[p, kc*WI + i] = x[idx[i], kc*128 + p]
            xt_f = np.zeros((WI, in_dim), np.float16)
            xt_f[:m] = x16[g][idx]
            xT[gl] = (
                xt_f.reshape(WI, KC, 128).transpose(2, 1, 0).reshape(
                    128, KC * WI
                )
            )
            # adjT[p, jc*WI + i] = adj[idx[i], idx[jc*128+p]]
            at = np.zeros((n_v, WI), np.float16)
            at[:m, :m] = adj01[g][np.ix_(idx, idx)].T
            adjTc[gl] = at.reshape(NV, 128, WI).transpose(1, 0, 2).reshape(
                128, NV * WI
            )
            srcv = np.zeros((WI, H))
            srcv[:m] = xc @ wa_src                     # [m, H]
            dstv = np.zeros((n_v, H))
            dstv[:m] = xc @ wa_dst
            for bi, (kind, h_) in enumerate(rep_blocks):
                if kind == "s":
                    v = srcv[:, h_]
                elif kind == "a":
                    v = np.exp(srcv[:, h_])
                else:
                    v = np.exp(0.2 * srcv[:, h_])
                rows[gl, 0, bi * WI : (bi + 1) * WI] = v.astype(np.float16)
            # dcall[p, c*H + h] = dst[c*128 + p, h]
            dcall = dstv.reshape(NV, 128, H).transpose(1, 0, 2).reshape(
                128, NV * H
            )
            small[gl, :, 0 : NV * H] = np.exp(dcall)
            small[gl, :, NV * H : 2 * NV * H] = np.exp(0.2 * dcall)
            small[gl, :, 2 * NV * H :] = dcall
        m_map = {
            "xT": xT,
            "adjT": adjTc,
            "wc": wc_full,
            "rows": rows,
            "small": small,
        }
        if not (np.all(gamma == 1.0) and np.all(beta == 0.0)):
            m_map["gamma_rep"] = np.ascontiguousarray(
                np.broadcast_to(gamma.astype(np.float32), (128, HD))
            )
            m_map["beta_rep"] = np.ascontiguousarray(
                np.broadcast_to(beta.astype(np.float32), (128, HD))
            )
        in_maps.append(m_map)
    return in_maps


def kernel(x, adj, mask, W, a_src, a_dst, gamma, beta, _trace=False):
    from concourse.bass_utils import run_bass_kernel_spmd

    b, n, in_dim = x.shape
    ng = b // NCORES
    trivial_ln = bool(np.all(gamma == 1.0) and np.all(beta == 0.0))

    idxs = [np.nonzero(mask[g] > 0)[0] for g in range(b)]
    max_m = max((len(i) for i in idxs), default=1)
    NV = max(1, -(-max_m // 128))
    WI = max(128, -(-max_m // 64) * 64)
    KC = in_dim // 128

    key = (ng, NV, WI, KC, trivial_ln, repr(sorted(CFG.items())))
    if key not in _PROG_CACHE:
        _PROG_CACHE[key] = _build_program(ng, NV, WI, KC, trivial_ln, CFG)
    nc = _PROG_CACHE[key]

    in_maps = _host_prep(
        x, adj, mask, W, a_src, a_dst, gamma, beta, ng, NV, WI, idxs
    )
    res = run_bass_kernel_spmd(
        nc, in_maps, core_ids=list(range(NCORES)), trace=_trace
    )
    full = np.zeros((b, n, HD), np.float32)
    if not trivial_ln:
        full[:] = beta.astype(np.float32)[None, None, :]
    for c in range(NCORES):
        o = res.results[c]["out"].reshape(ng, 128, NV, HD)
        o = o.transpose(0, 2, 1, 3).reshape(ng, NV * 128, HD)[:, :WI]
        for gl in range(ng):
            g = c * ng + gl
            idx = idxs[g]
            full[g, idx] = o[gl, : len(idx)].astype(np.float32)
    if _trace:
        return full, res
    return full


# revision 58
# speedup vs baseline: 1.0402x; 1.0338x over previous
# Dense GAT layer (4 heads, dim 64) on Trainium2 via Bass/Tile.
#
# Math: h = x@W; e_ij = LeakyReLU(src_i + dst_j, 0.2); masked softmax over j
# with valid = adj & mask_i & mask_j; out = LN((alpha @ h) * mask_i).
#
# Key ideas:
#   * Host-side node compaction: rows/cols with mask==0 contribute nothing
#     (their output is just beta); gather the valid nodes on host, run the
#     kernel on the compacted [m, m] problem (padded to a multiple of 128),
#     scatter back.  Cuts n^2 elementwise work ~2.5x and all DMA traffic.
#   * exp(LeakyReLU(t)) = max(exp(t), exp(0.2 t)),  t = src_i + dst_j
#     exp(src_i + dst_j) = exp(src_i) * exp(dst_j)   (rank-1 separable)
#   * "e^T" orientation: j (softmax axis) on partitions, i on the free axis,
#     so alpha@h needs no transposes and rowsum is a matmul ones-column.
#   * Per-head route split: some heads compute exp(Prelu(t)) on the ACT
#     engine (replicated src row in PSUM + per-partition dst bias), others
#     use the separable max form on the DVE.  adj-mask multiply placement is
#     tunable between DVE and GPSIMD.
#   * rstd for LayerNorm = exp(-0.5*ln(var+eps)) so every ACT function used
#     (exp/parametric_relu/ln/copy) lives in ONE activation table set
#     (natural_log_exp_and_others) -> no ~2.7us table reloads.
#   * adjT/xT pre-transposed on host -> no on-device DMA transposes.
# Sharding: data-parallel, 2 graphs per core across 8 cores.

import numpy as np

H, D = 4, 64
HD = H * D
EPS = 1e-5
NCORES = 8

_PROG_CACHE = {}

# Tuning knobs (baked into the compiled program; cache key includes them).
CFG = dict(
    na=(5, 5, 0, 0),        # per head: number of j-chunks on the ACT route
    head_order=(2, 0, 3, 1),
    mask_gps=(),            # adj-mul on GPSIMD ((h,jc) pairs)
    osb_act=(0, 1),         # heads whose 1/rowsum scale runs on ACT
    rep_cast_act=True,      # arep/crep PSUM->f16 casts on ACT (else DVE)
    hones_act=True,         # hones copy on ACT (else DVE)
    b4op=True,              # B route: ts+ts+max instead of ts+stt
)


def _build_program(ng, NV, WI, KC, trivial_ln, cfg):
    import concourse.bacc as bacc
    import concourse.mybir as mybir
    import concourse.tile as tile
    from concourse.bass import ts

    f16 = mybir.dt.float16
    f32 = mybir.dt.float32
    AF = mybir.ActivationFunctionType
    OP = mybir.AluOpType

    n_v = NV * 128          # padded j extent (partition chunks)
    E = D + 1               # head block in hones (64 h cols + 1 ones col)
    in_dim = KC * 128
    # i-axis chunk list: [start, width] with a partial last chunk (WI <= n_v)
    ichunks = [(s, min(128, WI - s)) for s in range(0, WI, 128)]
    assert len(ichunks) == NV

    na = cfg["na"]
    head_order = cfg["head_order"]
    mask_gps = set(cfg["mask_gps"])
    osb_act = set(cfg["osb_act"])
    # matmul moving-operand pieces (<=512 psum f32 columns per instruction)
    pieces = [(s, min(512, n_v - s)) for s in range(0, n_v, 512)]

    nc = bacc.Bacc()

    # rep row blocks packed per cfg: srow for A-heads, arow+crow for B-heads
    rep_blocks = []
    for h_ in cfg["head_order"]:
        if cfg["na"][h_] > 0:
            rep_blocks.append(("s", h_))
        if cfg["na"][h_] < NV:
            rep_blocks.append(("a", h_))
            rep_blocks.append(("c", h_))
    R = len(rep_blocks)

    xT = nc.dram_tensor("xT", [ng, 128, KC * WI], f16, kind="ExternalInput")
    adjT = nc.dram_tensor("adjT", [ng, 128, NV * WI], f16,
                          kind="ExternalInput")
    wc = nc.dram_tensor("wc", [128, KC * HD], f16, kind="ExternalInput")
    rows_d = nc.dram_tensor("rows", [ng, 1, R * WI], f16,
                            kind="ExternalInput")
    small_d = nc.dram_tensor("small", [ng, 128, 3 * NV * H], f32,
                             kind="ExternalInput")
    if not trivial_ln:
        gam = nc.dram_tensor("gamma_rep", [128, HD], f32, kind="ExternalInput")
        bet = nc.dram_tensor("beta_rep", [128, HD], f32, kind="ExternalInput")
    out = nc.dram_tensor("out", [ng, 128, NV * HD], f16,
                         kind="ExternalOutput")

    from contextlib import ExitStack

    with tile.TileContext(nc) as tc, ExitStack() as ctx:
        def pool(**kw):
            return ctx.enter_context(tc.tile_pool(**kw))

        big = WI > 640          # scale pipeline depths down for large n
        consts = pool(name="consts", bufs=1)
        xt_pool = pool(name="xt", bufs=2)
        adjt_pool = pool(name="adjt", bufs=2)
        reps_pool = pool(name="reps", bufs=1 if big else 3)
        hones_pool = pool(name="hones", bufs=NV + 2)
        small_pool = pool(name="small", bufs=4)
        ew_pool = pool(name="ew", bufs=4 if big else 8)
        houter0 = NV + 3 <= 8
        u_pool = pool(
            name="u",
            bufs=(3 * NV + 2) if not big else
                 (H * NV + 2) if not houter0 else (NV + 2),
        )
        osb_pool = pool(name="osb", bufs=NV + 2)
        ln_pool = pool(name="ln", bufs=4)
        misc_pool = pool(name="misc", bufs=2 if big else 3)
        # PSUM (8 banks of 2KB): ph + pav.  For NV <= 5 keep one pav tile
        # per i-chunk alive across the head loop (PE overlaps elementwise);
        # for larger NV fall back to transient pav tiles with all heads' u
        # tiles held in SBUF.
        houter = NV + 3 <= 8
        ph_pool = pool(name="ph", bufs=3 if houter else 2, space="PSUM")
        pav_pool = pool(name="pav", bufs=NV if houter else 3, space="PSUM")

        # ---- pin the ACT spline table set that covers Exp/Prelu/Ln/Copy,
        # so the compiler never inserts mid-kernel ~1.3us table reloads ----
        from concourse.hw_specs import get_activation_tables

        tabs = list(get_activation_tables(nc.m.arch).items())
        set_id = next(
            i for i, (nm, fs) in enumerate(tabs)
            if {AF.Exp, AF.Prelu, AF.Ln, AF.Copy} <= fs
        )
        nc.scalar.add_instruction(
            mybir.InstLoadActFuncSet(
                name=nc.get_next_instruction_name(),
                act_func_set_id=set_id,
                ins=[],
                outs=[],
            )
        )

        # ---- constants ----
        wc_sb = consts.tile([128, KC * HD], f16, tag="wc")
        nc.sync.dma_start(wc_sb[:], wc[:])
        if not trivial_ln:
            gam_sb = consts.tile([128, HD], f32, tag="gam")
            nc.sync.dma_start(gam_sb[:], gam[:])
            bet_sb = consts.tile([128, HD], f32, tag="bet")
            nc.sync.dma_start(bet_sb[:], bet[:])
        eps_sb = consts.tile([128, 1], f32, tag="eps")
        nc.vector.memset(eps_sb[:], EPS)

        need_a = any(na[h] > 0 for h in range(H))
        need_b = any(na[h] < NV for h in range(H))

        for g in range(ng):
            # ---- input DMAs (host pre-transposed; plain loads) ----
            # ---- batched input DMAs: one transfer each for the small
            # tables, x, the broadcast row-replicas, and the adjacency ----
            smalls = small_pool.tile([128, 3 * NV * H], f32, tag="smalls")
            nc.sync.dma_start(smalls[:], small_d[g])
            bmall = smalls[:, 0 : NV * H]
            dmall = smalls[:, NV * H : 2 * NV * H]
            dcall = smalls[:, 2 * NV * H : 3 * NV * H]
            # row replicas (partition-broadcast reads from HBM) come BEFORE
            # x: they gate the first elementwise ops, while x only feeds the
            # h-chunk pipeline whose results are needed later.  The first
            # head's blocks transfer separately so its ops start immediately.
            reps_all = reps_pool.tile([128, R * WI], f16, tag="reps")
            r1 = min(2, R)
            nc.sync.dma_start(
                reps_all[:, 0 : r1 * WI],
                rows_d[g, 0:1, 0 : r1 * WI].to_broadcast([128, r1 * WI]),
            )
            if R > r1:
                nc.sync.dma_start(
                    reps_all[:, r1 * WI :],
                    rows_d[g, 0:1, r1 * WI :].to_broadcast(
                        [128, (R - r1) * WI]
                    ),
                )
            xt_all = xt_pool.tile([128, KC * WI], f16, tag="xt")
            for kc in range(KC):
                nc.sync.dma_start(
                    xt_all[:, kc * WI : (kc + 1) * WI],
                    xT[g, :, kc * WI : (kc + 1) * WI],
                )
            xt = [xt_all[:, kc * WI : (kc + 1) * WI] for kc in range(KC)]
            sreps = {}
            areps = {}
            creps = {}
            for bi, (kind, h_) in enumerate(rep_blocks):
                v = reps_all[:, bi * WI : (bi + 1) * WI]
                (sreps if kind == "s" else areps if kind == "a" else creps)[
                    h_
                ] = v
            adjt_all = adjt_pool.tile([128, NV * WI], f16, tag="adjt")
            nc.sync.dma_start(adjt_all[:], adjT[g])
            adjt = [
                adjt_all[:, jc * WI : (jc + 1) * WI] for jc in range(NV)
            ]


            # ---- h_ext per chunk: hones (fp16 h + ones col), dst scalars ----
            # (after the rep matmuls so the serialized ph ring does not
            # head-of-line-block the PE queue for the replicates)
            hones = []
            for ic, (istart, icw) in enumerate(ichunks):
                ph = ph_pool.tile([128, HD], f32, tag="ph")
                for kc in range(KC):
                    nc.tensor.matmul(
                        ph[0:icw],
                        xt[kc][:, istart : istart + icw],
                        wc_sb[:, ts(kc, HD)],
                        start=(kc == 0),
                        stop=(kc == KC - 1),
                    )
                ho = hones_pool.tile([128, H * E], f16, tag="hones")
                ho3 = ho[:].rearrange("p (h e) -> p h e", h=H)
                if icw < 128:
                    # zero the padded j rows so pav contractions stay finite
                    nc.vector.memset(ho[icw:128, :], 0.0)
                if cfg["hones_act"]:
                    nc.scalar.copy(
                        ho3[0:icw, :, 0:D],
                        ph[0:icw].rearrange("p (h d) -> p h d", h=H),
                    )
                else:
                    nc.vector.tensor_copy(
                        ho3[0:icw, :, 0:D],
                        ph[0:icw].rearrange("p (h d) -> p h d", h=H),
                    )
                nc.vector.memset(ho3[0:icw, :, D : D + 1], 1.0)
                hones.append(ho)

            # ---- elementwise u tiles + per-head alpha@h accumulation ----
            o_sb = [
                osb_pool.tile([128, HD], f32, tag="osb", name=f"osb_{g}_{i}")
                for i in range(NV)
            ]
            o2all = misc_pool.tile(
                [128, NV * HD], f16, tag="o2all", name=f"o2all_{g}"
            )
            pav = None
            if houter:
                pav = [
                    pav_pool.tile(
                        [128, H * E], f32, tag="pav", name=f"pav_{g}_{i}"
                    )
                    for i in range(NV)
                ]
            u_save = {}
            for h in head_order:
                nah = na[h]
                a_jcs = list(range(nah))
                b_jcs = list(range(nah, NV))

                srep = sreps.get(h)
                arep = areps.get(h)
                crep = creps.get(h)

                u_tiles = [None] * NV
                for jc in a_jcs:
                    lrt = ew_pool.tile([128, WI], f16, tag="lrt")
                    nc.scalar.activation(
                        lrt[:], srep, AF.Prelu,
                        bias=dcall[:, jc * H + h : jc * H + h + 1],
                        alpha=0.2,
                    )
                    up = ew_pool.tile([128, WI], f16, tag="up")
                    nc.scalar.activation(up[:], lrt[:], AF.Exp)
                    u = u_pool.tile([128, WI], f16, tag="u")
                    meng = nc.gpsimd if (h, jc) in mask_gps else nc.vector
                    meng.tensor_mul(u[:], up[:], adjt[jc])
                    u_tiles[jc] = u[:]
                for jc in b_jcs:
                    t2 = ew_pool.tile([128, WI], f16, tag="t2")
                    nc.vector.tensor_scalar(
                        t2[:], crep,
                        dmall[:, jc * H + h : jc * H + h + 1], None,
                        op0=OP.mult,
                    )
                    w = ew_pool.tile([128, WI], f16, tag="w")
                    if cfg["b4op"]:
                        t1 = ew_pool.tile([128, WI], f16, tag="t1")
                        nc.vector.tensor_scalar(
                            t1[:], arep,
                            bmall[:, jc * H + h : jc * H + h + 1], None,
                            op0=OP.mult,
                        )
                        nc.vector.tensor_max(w[:], t1[:], t2[:])
                    else:
                        nc.vector.scalar_tensor_tensor(
                            w[:], arep,
                            bmall[:, jc * H + h : jc * H + h + 1], t2[:],
                            op0=OP.mult, op1=OP.max,
                        )
                    u = u_pool.tile([128, WI], f16, tag="u")
                    meng = nc.gpsimd if (h, jc) in mask_gps else nc.vector
                    meng.tensor_mul(u[:], w[:], adjt[jc])
                    u_tiles[jc] = u[:]

                if houter:
                    # alpha@h: this head's block of every chunk's PSUM tile,
                    # so the PE works during the elementwise phase
                    for ic, (istart, icw) in enumerate(ichunks):
                        for jc in range(NV):
                            nc.tensor.matmul(
                                pav[ic][0:icw, ts(h, E)],
                                u_tiles[jc][:, istart : istart + icw],
                                hones[jc][:, ts(h, E)],
                                start=(jc == 0),
                                stop=(jc == NV - 1),
                            )
                else:
                    for jc in range(NV):
                        u_save[(h, jc)] = u_tiles[jc]

            # ---- per-chunk softmax-normalize, LN stats, LN apply, store.
            # rstd = exp(-0.5 * ln(var + eps)); ln/exp share the table set
            # with Prelu/Exp above, so no ACT table reloads.
            for ic, (istart, icw) in enumerate(ichunks):
                if houter:
                    pav_t = pav[ic]
                else:
                    pav_t = pav_pool.tile([128, H * E], f32, tag="pav")
                    for h in range(H):
                        for jc in range(NV):
                            nc.tensor.matmul(
                                pav_t[0:icw, ts(h, E)],
                                u_save[(h, jc)][:, istart : istart + icw],
                                hones[jc][:, ts(h, E)],
                                start=(jc == 0),
                                stop=(jc == NV - 1),
                            )
                pav3 = pav_t[0:icw].rearrange("p (h e) -> p h e", h=H)
                rs4 = ln_pool.tile([128, H], f32, tag="rs4")
                nc.vector.reciprocal(rs4[0:icw], pav3[:, :, D])
                for hh in range(H):
                    if hh in osb_act:
                        nc.scalar.mul(
                            o_sb[ic][0:icw, ts(hh, D)],
                            pav3[:, hh, 0:D],
                            rs4[0:icw, hh : hh + 1],
                        )
                    else:
                        nc.vector.tensor_scalar(
                            o_sb[ic][0:icw, ts(hh, D)],
                            pav3[:, hh, 0:D],
                            rs4[0:icw, hh : hh + 1],
                            None,
                            op0=OP.mult,
                        )
                st6 = ln_pool.tile([128, 6], f32, tag="st6")
                nc.vector.bn_stats(st6[0:icw], o_sb[ic][0:icw])
                mv = ln_pool.tile([128, 2], f32, tag="mv")
                nc.vector.bn_aggr(mv[0:icw], st6[0:icw])
                rstd = ln_pool.tile([128, 2], f32, tag="rstd")
                nc.scalar.activation(
                    rstd[0:icw, 0:1], mv[0:icw, 1:2], AF.Ln,
                    bias=eps_sb[0:icw],
                )
                nc.scalar.activation(
                    rstd[0:icw, 1:2], rstd[0:icw, 0:1], AF.Exp, scale=-0.5
                )
                nmr = ln_pool.tile([128, 1], f32, tag="nmr")
                nc.vector.scalar_tensor_tensor(
                    nmr[0:icw], mv[0:icw, 0:1], -1.0, rstd[0:icw, 1:2],
                    op0=OP.mult, op1=OP.mult,
                )
                if trivial_ln:
                    nc.scalar.activation(
                        o2all[0:icw, ts(ic, HD)],
                        o_sb[ic][0:icw],
                        AF.Identity,
                        bias=nmr[0:icw],
                        scale=rstd[0:icw, 1:2],
                    )
                else:
                    o3 = misc_pool.tile([128, HD], f32, tag="o3")
                    nc.scalar.activation(
                        o3[0:icw],
                        o_sb[ic][0:icw],
                        AF.Identity,
                        bias=nmr[0:icw],
                        scale=rstd[0:icw, 1:2],
                    )
                    nc.vector.tensor_mul(o3[0:icw], o3[0:icw], gam_sb[0:icw])
                    nc.vector.tensor_add(
                        o2all[0:icw, ts(ic, HD)], o3[0:icw], bet_sb[0:icw]
                    )
                nc.gpsimd.dma_start(
                    out[g, 0:icw, ic * HD : (ic + 1) * HD],
                    o2all[0:icw, ts(ic, HD)],
                )


    nc.compile()
    return nc


def _host_prep(x, adj, mask, W, a_src, a_dst, gamma, beta, ng, NV, WI, idxs):
    """Per-core input maps: compaction + dtype packing + weight folding."""
    b, n, in_dim = x.shape
    KC = in_dim // 128
    n_v = NV * 128

    # Fold attention vectors into W:  Wa[c, h] = sum_d W[c, h*D+d] * a[h, d]
    Wr = W.astype(np.float64).reshape(in_dim, H, D)
    wa_src = np.einsum("chd,hd->ch", Wr, a_src.astype(np.float64))
    wa_dst = np.einsum("chd,hd->ch", Wr, a_dst.astype(np.float64))

    wc_full = np.ascontiguousarray(
        W.astype(np.float16).reshape(KC, 128, HD).transpose(1, 0, 2)
    ).reshape(128, KC * HD)
    x16 = x.astype(np.float16)
    adj01 = adj != 0
    NV = n_v // 128

    # rep row packing must mirror _build_program's rep_blocks
    rep_blocks = []
    for h_ in CFG["head_order"]:
        if CFG["na"][h_] > 0:
            rep_blocks.append(("s", h_))
        if CFG["na"][h_] < NV:
            rep_blocks.append(("a", h_))
            rep_blocks.append(("c", h_))
    R = len(rep_blocks)

    in_maps = []
    for c in range(NCORES):
        xT = np.zeros((ng, 128, KC * WI), np.float16)
        adjTc = np.zeros((ng, 128, NV * WI), np.float16)
        rows = np.zeros((ng, 1, R * WI), np.float16)
        small = np.zeros((ng, 128, 3 * NV * H), np.float32)
        for gl in range(ng):
            g = c * ng + gl
            idx = idxs[g]
            m = len(idx)
            xc = x[g][idx].astype(np.float64)          # [m, in_dim]
            # xT[p, kc*WI + i] = x[idx[i], kc*128 + p]
            xt_f = np.zeros((WI, in_dim), np.float16)
            xt_f[:m] = x16[g][idx]
            xT[gl] = (
                xt_f.reshape(WI, KC, 128).transpose(2, 1, 0).reshape(
                    128, KC * WI
                )
            )
            # adjT[p, jc*WI + i] = adj[idx[i], idx[jc*128+p]]
            at = np.zeros((n_v, WI), np.float16)
            at[:m, :m] = adj01[g][np.ix_(idx, idx)].T
            adjTc[gl] = at.reshape(NV, 128, WI).transpose(1, 0, 2).reshape(
                128, NV * WI
            )
            srcv = np.zeros((WI, H))
            srcv[:m] = xc @ wa_src                     # [m, H]
            dstv = np.zeros((n_v, H))
            dstv[:m] = xc @ wa_dst
            for bi, (kind, h_) in enumerate(rep_blocks):
                if kind == "s":
                    v = srcv[:, h_]
                elif kind == "a":
                    v = np.exp(srcv[:, h_])
                else:
                    v = np.exp(0.2 * srcv[:, h_])
                rows[gl, 0, bi * WI : (bi + 1) * WI] = v.astype(np.float16)
            # dcall[p, c*H + h] = dst[c*128 + p, h]
            dcall = dstv.reshape(NV, 128, H).transpose(1, 0, 2).reshape(
                128, NV * H
            )
            small[gl, :, 0 : NV * H] = np.exp(dcall)
            small[gl, :, NV * H : 2 * NV * H] = np.exp(0.2 * dcall)
            small[gl, :, 2 * NV * H :] = dcall
        m_map = {
            "xT": xT,
            "adjT": adjTc,
            "wc": wc_full,
            "rows": rows,
            "small": small,
        }
        if not (np.all(gamma == 1.0) and np.all(beta == 0.0)):
            m_map["gamma_rep"] = np.ascontiguousarray(
                np.broadcast_to(gamma.astype(np.float32), (128, HD))
            )
            m_map["beta_rep"] = np.ascontiguousarray(
                np.broadcast_to(beta.astype(np.float32), (128, HD))
            )
        in_maps.append(m_map)
    return in_maps


def kernel(x, adj, mask, W, a_src, a_dst, gamma, beta, _trace=False):
    from concourse.bass_utils import run_bass_kernel_spmd

    b, n, in_dim = x.shape
    ng = b // NCORES
    trivial_ln = bool(np.all(gamma == 1.0) and np.all(beta == 0.0))

    idxs = [np.nonzero(mask[g] > 0)[0] for g in range(b)]
    max_m = max((len(i) for i in idxs), default=1)
    NV = max(1, -(-max_m // 128))
    WI = max(128, -(-max_m // 64) * 64)
    KC = in_dim // 128

    key = (ng, NV, WI, KC, trivial_ln, repr(sorted(CFG.items())))
    if key not in _PROG_CACHE:
        _PROG_CACHE[key] = _build_program(ng, NV, WI, KC, trivial_ln, CFG)
    nc = _PROG_CACHE[key]

    in_maps = _host_prep(
        x, adj, mask, W, a_src, a_dst, gamma, beta, ng, NV, WI, idxs
    )
    res = run_bass_kernel_spmd(
        nc, in_maps, core_ids=list(range(NCORES)), trace=_trace
    )
    full = np.zeros((b, n, HD), np.float32)
    if not trivial_ln:
        full[:] = beta.astype(np.float32)[None, None, :]
    for c in range(NCORES):
        o = res.results[c]["out"].reshape(ng, 128, NV, HD)
        o = o.transpose(0, 2, 1, 3).reshape(ng, NV * 128, HD)[:, :WI]
        for gl in range(ng):
            g = c * ng + gl
            idx = idxs[g]
            full[g, idx] = o[gl, : len(idx)].astype(np.float32)
    if _trace:
        return full, res
    return full


# revision 59
# speedup vs baseline: 1.0490x; 1.0084x over previous
# Dense GAT layer (4 heads, dim 64) on Trainium2 via Bass/Tile.
#
# Math: h = x@W; e_ij = LeakyReLU(src_i + dst_j, 0.2); masked softmax over j
# with valid = adj & mask_i & mask_j; out = LN((alpha @ h) * mask_i).
#
# Key ideas:
#   * Host-side node compaction: rows/cols with mask==0 contribute nothing
#     (their output is just beta); gather the valid nodes on host, run the
#     kernel on the compacted [m, m] problem (padded to a multiple of 128),
#     scatter back.  Cuts n^2 elementwise work ~2.5x and all DMA traffic.
#   * exp(LeakyReLU(t)) = max(exp(t), exp(0.2 t)),  t = src_i + dst_j
#     exp(src_i + dst_j) = exp(src_i) * exp(dst_j)   (rank-1 separable)
#   * "e^T" orientation: j (softmax axis) on partitions, i on the free axis,
#     so alpha@h needs no transposes and rowsum is a matmul ones-column.
#   * Per-head route split: some heads compute exp(Prelu(t)) on the ACT
#     engine (replicated src row in PSUM + per-partition dst bias), others
#     use the separable max form on the DVE.  adj-mask multiply placement is
#     tunable between DVE and GPSIMD.
#   * rstd for LayerNorm = exp(-0.5*ln(var+eps)) so every ACT function used
#     (exp/parametric_relu/ln/copy) lives in ONE activation table set
#     (natural_log_exp_and_others) -> no ~2.7us table reloads.
#   * adjT/xT pre-transposed on host -> no on-device DMA transposes.
# Sharding: data-parallel, 2 graphs per core across 8 cores.

import numpy as np

H, D = 4, 64
HD = H * D
EPS = 1e-5
NCORES = 8

_PROG_CACHE = {}

# Tuning knobs (baked into the compiled program; cache key includes them).
CFG = dict(
    na=(5, 5, 0, 0),        # per head: number of j-chunks on the ACT route
    head_order=(2, 0, 3, 1),
    mask_gps=(),            # adj-mul on GPSIMD ((h,jc) pairs)
    osb_act=(0, 1),         # heads whose 1/rowsum scale runs on ACT
    rep_cast_act=True,      # arep/crep PSUM->f16 casts on ACT (else DVE)
    hones_act=True,         # hones copy on ACT (else DVE)
    b4op=True,              # B route: ts+ts+max instead of ts+stt
)


def _build_program(ng, NV, WI, KC, trivial_ln, cfg):
    import concourse.bacc as bacc
    import concourse.mybir as mybir
    import concourse.tile as tile
    from concourse.bass import ts

    f16 = mybir.dt.float16
    f32 = mybir.dt.float32
    AF = mybir.ActivationFunctionType
    OP = mybir.AluOpType

    n_v = NV * 128          # padded j extent (partition chunks)
    E = D + 1               # head block in hones (64 h cols + 1 ones col)
    in_dim = KC * 128
    # i-axis chunk list: [start, width] with a partial last chunk (WI <= n_v)
    ichunks = [(s, min(128, WI - s)) for s in range(0, WI, 128)]
    assert len(ichunks) == NV

    na = cfg["na"]
    head_order = cfg["head_order"]
    mask_gps = set(cfg["mask_gps"])
    osb_act = set(cfg["osb_act"])
    # matmul moving-operand pieces (<=512 psum f32 columns per instruction)
    pieces = [(s, min(512, n_v - s)) for s in range(0, n_v, 512)]

    nc = bacc.Bacc()

    # rep row blocks packed per cfg: srow for A-heads, arow+crow for B-heads
    rep_blocks = []
    for h_ in cfg["head_order"]:
        if cfg["na"][h_] > 0:
            rep_blocks.append(("s", h_))
        if cfg["na"][h_] < NV:
            rep_blocks.append(("a", h_))
            rep_blocks.append(("c", h_))
    R = len(rep_blocks)

    xT = nc.dram_tensor("xT", [ng, 128, KC * WI], f16, kind="ExternalInput")
    adjT = nc.dram_tensor("adjT", [ng, 128, NV * WI], f16,
                          kind="ExternalInput")
    wc = nc.dram_tensor("wc", [128, KC * HD], f16, kind="ExternalInput")
    rows_d = nc.dram_tensor("rows", [ng, 1, R * WI], f16,
                            kind="ExternalInput")
    small_d = nc.dram_tensor("small", [ng, 128, 3 * NV * H], f32,
                             kind="ExternalInput")
    if not trivial_ln:
        gam = nc.dram_tensor("gamma_rep", [128, HD], f32, kind="ExternalInput")
        bet = nc.dram_tensor("beta_rep", [128, HD], f32, kind="ExternalInput")
    out = nc.dram_tensor("out", [ng, 128, NV * HD], f16,
                         kind="ExternalOutput")

    from contextlib import ExitStack

    with tile.TileContext(nc) as tc, ExitStack() as ctx:
        def pool(**kw):
            return ctx.enter_context(tc.tile_pool(**kw))

        big = WI > 640          # scale pipeline depths down for large n
        consts = pool(name="consts", bufs=1)
        xt_pool = pool(name="xt", bufs=2)
        adjt_pool = pool(name="adjt", bufs=2)
        reps_pool = pool(name="reps", bufs=1 if big else 3)
        hones_pool = pool(name="hones", bufs=NV + 2)
        small_pool = pool(name="small", bufs=4)
        ew_pool = pool(name="ew", bufs=4 if big else 8)
        houter0 = NV + 3 <= 8
        u_pool = pool(
            name="u",
            bufs=(3 * NV + 2) if not big else
                 (H * NV + 2) if not houter0 else (NV + 2),
        )
        osb_pool = pool(name="osb", bufs=NV + 2)
        ln_pool = pool(name="ln", bufs=4)
        misc_pool = pool(name="misc", bufs=2 if big else 3)
        # PSUM (8 banks of 2KB): ph + pav.  For NV <= 5 keep one pav tile
        # per i-chunk alive across the head loop (PE overlaps elementwise);
        # for larger NV fall back to transient pav tiles with all heads' u
        # tiles held in SBUF.
        houter = NV + 3 <= 8
        ph_pool = pool(name="ph", bufs=3 if houter else 2, space="PSUM")
        pav_pool = pool(name="pav", bufs=NV if houter else 3, space="PSUM")

        # ---- pin the ACT spline table set that covers Exp/Prelu/Ln/Copy,
        # so the compiler never inserts mid-kernel ~1.3us table reloads ----
        from concourse.hw_specs import get_activation_tables

        tabs = list(get_activation_tables(nc.m.arch).items())
        set_id = next(
            i for i, (nm, fs) in enumerate(tabs)
            if {AF.Exp, AF.Prelu, AF.Ln, AF.Copy} <= fs
        )
        nc.scalar.add_instruction(
            mybir.InstLoadActFuncSet(
                name=nc.get_next_instruction_name(),
                act_func_set_id=set_id,
                ins=[],
                outs=[],
            )
        )

        # ---- constants ----
        wc_sb = consts.tile([128, KC * HD], f16, tag="wc")
        nc.sync.dma_start(wc_sb[:], wc[:])
        if not trivial_ln:
            gam_sb = consts.tile([128, HD], f32, tag="gam")
            nc.sync.dma_start(gam_sb[:], gam[:])
            bet_sb = consts.tile([128, HD], f32, tag="bet")
            nc.sync.dma_start(bet_sb[:], bet[:])
        eps_sb = consts.tile([128, 1], f32, tag="eps")
        nc.vector.memset(eps_sb[:], EPS)

        need_a = any(na[h] > 0 for h in range(H))
        need_b = any(na[h] < NV for h in range(H))

        for g in range(ng):
            # ---- input DMAs (host pre-transposed; plain loads) ----
            # ---- batched input DMAs: one transfer each for the small
            # tables, x, the broadcast row-replicas, and the adjacency ----
            smalls = small_pool.tile([128, 3 * NV * H], f32, tag="smalls")
            nc.sync.dma_start(smalls[:], small_d[g])
            bmall = smalls[:, 0 : NV * H]
            dmall = smalls[:, NV * H : 2 * NV * H]
            dcall = smalls[:, 2 * NV * H : 3 * NV * H]
            # row replicas (partition-broadcast reads from HBM) come BEFORE
            # x: they gate the first elementwise ops, while x only feeds the
            # h-chunk pipeline whose results are needed later.  The first
            # head's blocks transfer separately so its ops start immediately.
            reps_all = reps_pool.tile([128, R * WI], f16, tag="reps")
            r1 = min(2, R)
            nc.sync.dma_start(
                reps_all[:, 0 : r1 * WI],
                rows_d[g, 0:1, 0 : r1 * WI].to_broadcast([128, r1 * WI]),
            )
            if R > r1:
                nc.sync.dma_start(
                    reps_all[:, r1 * WI :],
                    rows_d[g, 0:1, r1 * WI :].to_broadcast(
                        [128, (R - r1) * WI]
                    ),
                )
            xt_all = xt_pool.tile([128, KC * WI], f16, tag="xt")
            for kc in range(KC):
                nc.sync.dma_start(
                    xt_all[:, kc * WI : (kc + 1) * WI],
                    xT[g, :, kc * WI : (kc + 1) * WI],
                )
            xt = [xt_all[:, kc * WI : (kc + 1) * WI] for kc in range(KC)]
            sreps = {}
            areps = {}
            creps = {}
            for bi, (kind, h_) in enumerate(rep_blocks):
                v = reps_all[:, bi * WI : (bi + 1) * WI]
                (sreps if kind == "s" else areps if kind == "a" else creps)[
                    h_
                ] = v
            adjt_all = adjt_pool.tile([128, NV * WI], f16, tag="adjt")
            asplit = min(2, NV) * WI
            nc.sync.dma_start(adjt_all[:, 0:asplit], adjT[g, :, 0:asplit])
            nc.sync.dma_start(adjt_all[:, asplit:], adjT[g, :, asplit:])
            adjt = [
                adjt_all[:, jc * WI : (jc + 1) * WI] for jc in range(NV)
            ]


            # ---- h_ext per chunk: hones (fp16 h + ones col), dst scalars ----
            # (after the rep matmuls so the serialized ph ring does not
            # head-of-line-block the PE queue for the replicates)
            hones = []
            for ic, (istart, icw) in enumerate(ichunks):
                ph = ph_pool.tile([128, HD], f32, tag="ph")
                for kc in range(KC):
                    nc.tensor.matmul(
                        ph[0:icw],
                        xt[kc][:, istart : istart + icw],
                        wc_sb[:, ts(kc, HD)],
                        start=(kc == 0),
                        stop=(kc == KC - 1),
                    )
                ho = hones_pool.tile([128, H * E], f16, tag="hones")
                ho3 = ho[:].rearrange("p (h e) -> p h e", h=H)
                if icw < 128:
                    # zero the padded j rows so pav contractions stay finite
                    nc.vector.memset(ho[icw:128, :], 0.0)
                if cfg["hones_act"]:
                    nc.scalar.copy(
                        ho3[0:icw, :, 0:D],
                        ph[0:icw].rearrange("p (h d) -> p h d", h=H),
                    )
                else:
                    nc.vector.tensor_copy(
                        ho3[0:icw, :, 0:D],
                        ph[0:icw].rearrange("p (h d) -> p h d", h=H),
                    )
                nc.vector.memset(ho3[0:icw, :, D : D + 1], 1.0)
                hones.append(ho)

            # ---- elementwise u tiles + per-head alpha@h accumulation ----
            o_sb = [
                osb_pool.tile([128, HD], f32, tag="osb", name=f"osb_{g}_{i}")
                for i in range(NV)
            ]
            o2all = misc_pool.tile(
                [128, NV * HD], f16, tag="o2all", name=f"o2all_{g}"
            )
            pav = None
            if houter:
                pav = [
                    pav_pool.tile(
                        [128, H * E], f32, tag="pav", name=f"pav_{g}_{i}"
                    )
                    for i in range(NV)
                ]
            u_save = {}
            for h in head_order:
                nah = na[h]
                a_jcs = list(range(nah))
                b_jcs = list(range(nah, NV))

                srep = sreps.get(h)
                arep = areps.get(h)
                crep = creps.get(h)

                u_tiles = [None] * NV
                for jc in a_jcs:
                    lrt = ew_pool.tile([128, WI], f16, tag="lrt")
                    nc.scalar.activation(
                        lrt[:], srep, AF.Prelu,
                        bias=dcall[:, jc * H + h : jc * H + h + 1],
                        alpha=0.2,
                    )
                    up = ew_pool.tile([128, WI], f16, tag="up")
                    nc.scalar.activation(up[:], lrt[:], AF.Exp)
                    u = u_pool.tile([128, WI], f16, tag="u")
                    meng = nc.gpsimd if (h, jc) in mask_gps else nc.vector
                    meng.tensor_mul(u[:], up[:], adjt[jc])
                    u_tiles[jc] = u[:]
                for jc in b_jcs:
                    t2 = ew_pool.tile([128, WI], f16, tag="t2")
                    nc.vector.tensor_scalar(
                        t2[:], crep,
                        dmall[:, jc * H + h : jc * H + h + 1], None,
                        op0=OP.mult,
                    )
                    w = ew_pool.tile([128, WI], f16, tag="w")
                    if cfg["b4op"]:
                        t1 = ew_pool.tile([128, WI], f16, tag="t1")
                        nc.vector.tensor_scalar(
                            t1[:], arep,
                            bmall[:, jc * H + h : jc * H + h + 1], None,
                            op0=OP.mult,
                        )
                        nc.vector.tensor_max(w[:], t1[:], t2[:])
                    else:
                        nc.vector.scalar_tensor_tensor(
                            w[:], arep,
                            bmall[:, jc * H + h : jc * H + h + 1], t2[:],
                            op0=OP.mult, op1=OP.max,
                        )
                    u = u_pool.tile([128, WI], f16, tag="u")
                    meng = nc.gpsimd if (h, jc) in mask_gps else nc.vector
                    meng.tensor_mul(u[:], w[:], adjt[jc])
                    u_tiles[jc] = u[:]

                if houter:
                    # alpha@h: this head's block of every chunk's PSUM tile,
                    # so the PE works during the elementwise phase
                    for ic, (istart, icw) in enumerate(ichunks):
                        for jc in range(NV):
                            nc.tensor.matmul(
                                pav[ic][0:icw, ts(h, E)],
                                u_tiles[jc][:, istart : istart + icw],
                                hones[jc][:, ts(h, E)],
                                start=(jc == 0),
                                stop=(jc == NV - 1),
                            )
                else:
                    for jc in range(NV):
                        u_save[(h, jc)] = u_tiles[jc]

            # ---- per-chunk softmax-normalize, LN stats, LN apply, store.
            # rstd = exp(-0.5 * ln(var + eps)); ln/exp share the table set
            # with Prelu/Exp above, so no ACT table reloads.
            for ic, (istart, icw) in enumerate(ichunks):
                if houter:
                    pav_t = pav[ic]
                else:
                    pav_t = pav_pool.tile([128, H * E], f32, tag="pav")
                    for h in range(H):
                        for jc in range(NV):
                            nc.tensor.matmul(
                                pav_t[0:icw, ts(h, E)],
                                u_save[(h, jc)][:, istart : istart + icw],
                                hones[jc][:, ts(h, E)],
                                start=(jc == 0),
                                stop=(jc == NV - 1),
                            )
                pav3 = pav_t[0:icw].rearrange("p (h e) -> p h e", h=H)
                rs4 = ln_pool.tile([128, H], f32, tag="rs4")
                nc.vector.reciprocal(rs4[0:icw], pav3[:, :, D])
                for hh in range(H):
                    if hh in osb_act:
                        nc.scalar.mul(
                            o_sb[ic][0:icw, ts(hh, D)],
                            pav3[:, hh, 0:D],
                            rs4[0:icw, hh : hh + 1],
                        )
                    else:
                        nc.vector.tensor_scalar(
                            o_sb[ic][0:icw, ts(hh, D)],
                            pav3[:, hh, 0:D],
                            rs4[0:icw, hh : hh + 1],
                            None,
                            op0=OP.mult,
                        )
                st6 = ln_pool.tile([128, 6], f32, tag="st6")
                nc.vector.bn_stats(st6[0:icw], o_sb[ic][0:icw])
                mv = ln_pool.tile([128, 2], f32, tag="mv")
                nc.vector.bn_aggr(mv[0:icw], st6[0:icw])
                rstd = ln_pool.tile([128, 2], f32, tag="rstd")
                nc.scalar.activation(
                    rstd[0:icw, 0:1], mv[0:icw, 1:2], AF.Ln,
                    bias=eps_sb[0:icw],
                )
                nc.scalar.activation(
                    rstd[0:icw, 1:2], rstd[0:icw, 0:1], AF.Exp, scale=-0.5
                )
                nmr = ln_pool.tile([128, 1], f32, tag="nmr")
                nc.vector.scalar_tensor_tensor(
                    nmr[0:icw], mv[0:icw, 0:1], -1.0, rstd[0:icw, 1:2],
                    op0=OP.mult, op1=OP.mult,
                )
                if trivial_ln:
                    nc.scalar.activation(
                        o2all[0:icw, ts(ic, HD)],
                        o_sb[ic][0:icw],
                        AF.Identity,
                        bias=nmr[0:icw],
                        scale=rstd[0:icw, 1:2],
                    )
                else:
                    o3 = misc_pool.tile([128, HD], f32, tag="o3")
                    nc.scalar.activation(
                        o3[0:icw],
                        o_sb[ic][0:icw],
                        AF.Identity,
                        bias=nmr[0:icw],
                        scale=rstd[0:icw, 1:2],
                    )
                    nc.vector.tensor_mul(o3[0:icw], o3[0:icw], gam_sb[0:icw])
                    nc.vector.tensor_add(
                        o2all[0:icw, ts(ic, HD)], o3[0:icw], bet_sb[0:icw]
                    )
                nc.gpsimd.dma_start(
                    out[g, 0:icw, ic * HD : (ic + 1) * HD],
                    o2all[0:icw, ts(ic, HD)],
                )


    nc.compile()
    return nc


def _host_prep(x, adj, mask, W, a_src, a_dst, gamma, beta, ng, NV, WI, idxs):
    """Per-core input maps: compaction + dtype packing + weight folding."""
    b, n, in_dim = x.shape
    KC = in_dim // 128
    n_v = NV * 128

    # Fold attention vectors into W:  Wa[c, h] = sum_d W[c, h*D+d] * a[h, d]
    Wr = W.astype(np.float64).reshape(in_dim, H, D)
    wa_src = np.einsum("chd,hd->ch", Wr, a_src.astype(np.float64))
    wa_dst = np.einsum("chd,hd->ch", Wr, a_dst.astype(np.float64))

    wc_full = np.ascontiguousarray(
        W.astype(np.float16).reshape(KC, 128, HD).transpose(1, 0, 2)
    ).reshape(128, KC * HD)
    x16 = x.astype(np.float16)
    adj01 = adj != 0
    NV = n_v // 128

    # rep row packing must mirror _build_program's rep_blocks
    rep_blocks = []
    for h_ in CFG["head_order"]:
        if CFG["na"][h_] > 0:
            rep_blocks.append(("s", h_))
        if CFG["na"][h_] < NV:
            rep_blocks.append(("a", h_))
            rep_blocks.append(("c", h_))
    R = len(rep_blocks)

    in_maps = []
    for c in range(NCORES):
        xT = np.zeros((ng, 128, KC * WI), np.float16)
        adjTc = np.zeros((ng, 128, NV * WI), np.float16)
        rows = np.zeros((ng, 1, R * WI), np.float16)
        small = np.zeros((ng, 128, 3 * NV * H), np.float32)
        for gl in range(ng):
            g = c * ng + gl
            idx = idxs[g]
            m = len(idx)
            xc = x[g][idx].astype(np.float64)          # [m, in_dim]
            # xT[p, kc*WI + i] = x[idx[i], kc*128 + p]
            xt_f = np.zeros((WI, in_dim), np.float16)
            xt_f[:m] = x16[g][idx]
            xT[gl] = (
                xt_f.reshape(WI, KC, 128).transpose(2, 1, 0).reshape(
                    128, KC * WI
                )
            )
            # adjT[p, jc*WI + i] = adj[idx[i], idx[jc*128+p]]
            at = np.zeros((n_v, WI), np.float16)
            at[:m, :m] = adj01[g][np.ix_(idx, idx)].T
            adjTc[gl] = at.reshape(NV, 128, WI).transpose(1, 0, 2).reshape(
                128, NV * WI
            )
            srcv = np.zeros((WI, H))
            srcv[:m] = xc @ wa_src                     # [m, H]
            dstv = np.zeros((n_v, H))
            dstv[:m] = xc @ wa_dst
            for bi, (kind, h_) in enumerate(rep_blocks):
                if kind == "s":
                    v = srcv[:, h_]
                elif kind == "a":
                    v = np.exp(srcv[:, h_])
                else:
                    v = np.exp(0.2 * srcv[:, h_])
                rows[gl, 0, bi * WI : (bi + 1) * WI] = v.astype(np.float16)
            # dcall[p, c*H + h] = dst[c*128 + p, h]
            dcall = dstv.reshape(NV, 128, H).transpose(1, 0, 2).reshape(
                128, NV * H
            )
            small[gl, :, 0 : NV * H] = np.exp(dcall)
            small[gl, :, NV * H : 2 * NV * H] = np.exp(0.2 * dcall)
            small[gl, :, 2 * NV * H :] = dcall
        m_map = {
            "xT": xT,
            "adjT": adjTc,
            "wc": wc_full,
            "rows": rows,
            "small": small,
        }
        if not (np.all(gamma == 1.0) and np.all(beta == 0.0)):
            m_map["gamma_rep"] = np.ascontiguousarray(
                np.broadcast_to(gamma.astype(np.float32), (128, HD))
            )
            m_map["beta_rep"] = np.ascontiguousarray(
                np.broadcast_to(beta.astype(np.float32), (128, HD))
            )
        in_maps.append(m_map)
    return in_maps


def kernel(x, adj, mask, W, a_src, a_dst, gamma, beta, _trace=False):
    from concourse.bass_utils import run_bass_kernel_spmd

    b, n, in_dim = x.shape
    ng = b // NCORES
    trivial_ln = bool(np.all(gamma == 1.0) and np.all(beta == 0.0))

    idxs = [np.nonzero(mask[g] > 0)[0] for g in range(b)]
    max_m = max((len(i) for i in idxs), default=1)
    NV = max(1, -(-max_m // 128))
    WI = max(128, -(-max_m // 64) * 64)
    KC = in_dim // 128

    key = (ng, NV, WI, KC, trivial_ln, repr(sorted(CFG.items())))
    if key not in _PROG_CACHE:
        _PROG_CACHE[key] = _build_program(ng, NV, WI, KC, trivial_ln, CFG)
    nc = _PROG_CACHE[key]

    in_maps = _host_prep(
        x, adj, mask, W, a_src, a_dst, gamma, beta, ng, NV, WI, idxs
    )
    res = run_bass_kernel_spmd(
        nc, in_maps, core_ids=list(range(NCORES)), trace=_trace
    )
    full = np.zeros((b, n, HD), np.float32)
    if not trivial_ln:
        full[:] = beta.astype(np.float32)[None, None, :]
    for c in range(NCORES):
        o = res.results[c]["out"].reshape(ng, 128, NV, HD)
        o = o.transpose(0, 2, 1, 3).reshape(ng, NV * 128, HD)[:, :WI]
        for gl in range(ng):
            g = c * ng + gl
            idx = idxs[g]
            full[g, idx] = o[gl, : len(idx)].astype(np.float32)
    if _trace:
        return full, res
    return full
